# revision 13
# baseline (speedup 1.0000x reference)
"""Per-pixel depthwise 3x3 conv (Conv2dLocal) on 8 Trainium2 NeuronCores.

out[b,c,h,w] = sum_{i,j in 3x3} x[b,c,h+i-1,w+j-1] * weight[b, c*9+3i+j, h, w]

Sharding: 8 cores = 2 batches x 4 H-slabs of 64 rows (data/spatial parallel).
The host pads the input spatially (1-px halo on H and W) and hands every core
an overlapping x slab, so the device program is identical and branch-free on
all cores (pure SPMD, no collectives).

Per-core layout: partition p = hb*32 + c (hb: 16-row block 0..3, c: channel);
free dim = (row, w), so all nine 3x3 tap shifts are free-dim offsets into a
single resident x slab [128, 18, 514].

DMA: the host pre-permutes the weight slab to [tap, grp, hb, c, r, w] so each
(tap, row-group) tile is one contiguous 1 MiB DRAM block — scattered-source
DMAs stripe over only 4 of the 16 SDMA engines (~110 GB/s), contiguous ones
use all 16 (~340 GB/s). Output uses a device-friendly contiguous layout too,
unscrambled on the host. Weights stream on the sync HWDGE ring; x and outputs
ride the scalar ring so they never queue behind the weight stream.

Compute: 9 DVE multiplies per group (fp32 tensor_tensor, 1x mode — uses only
DVE's dedicated SBUF port pair, so GpSimd runs truly in parallel) and the 8-way
tap accumulation split DVE/GpSimd. No PE/PSUM: fp32 matmul streams at 1/4 rate,
so identity-matmul accumulation is slower than DVE adds.
"""

import sys

if "/opt/trn_rl_repo" not in sys.path:
    sys.path.insert(0, "/opt/trn_rl_repo")

from contextlib import ExitStack

import numpy as np

import concourse.mybir as mybir
import concourse.tile as tile
from concourse import bacc
from concourse.bass_utils import run_bass_kernel_spmd

# Problem shape (hardcoded per harness contract)
B, C, H, W = 2, 32, 256, 512
K = 3
KK = K * K
N_CORES = 8

# Per-core decomposition
HL = H // 4          # 64 local rows per core
HB = 4               # row-blocks per core (partition groups)
RB = HL // HB        # 16 rows per partition
G = 4                # rows processed per group
NGRP = RB // G       # 4 groups
WP = W + 2           # width incl. halo
NP = 128             # partitions

FP32 = mybir.dt.float32
ADD = mybir.AluOpType.add
MULT = mybir.AluOpType.mult

_PROGRAM = None


def _build_program() -> bacc.Bacc:
    nc = bacc.Bacc(
        "TRN2", target_bir_lowering=False, debug=False, num_devices=N_CORES
    )
    x_d = nc.declare_dram_parameter("x", [HB, C, RB + 2, WP], FP32, isOutput=False)
    w_d = nc.declare_dram_parameter(
        "w", [KK, NGRP, HB, C, G, W], FP32, isOutput=False
    )
    o_d = nc.declare_dram_parameter("o", [NGRP, HB, C, G, W], FP32, isOutput=True)

    HW = W // 2      # half-panel width
    HWP = HW + 2     # half-panel incl. w halo

    with tile.TileContext(nc) as tc, ExitStack() as ctx:
        x_pool = ctx.enter_context(tc.tile_pool(name="x", bufs=1))
        w_pool = ctx.enter_context(tc.tile_pool(name="wt", bufs=12))
        prod_pool = ctx.enter_context(tc.tile_pool(name="prod", bufs=6))
        acc_pool = ctx.enter_context(tc.tile_pool(name="acc", bufs=4))
        out_pool = ctx.enter_context(tc.tile_pool(name="outsb", bufs=2))
        # half-panel x windows: [128, 6, 258] fp32 = 4 PSUM banks, so two
        # buffers ping-pong across the full 8-bank PSUM
        xps_pool = ctx.enter_context(tc.tile_pool(name="xps", bufs=2, space="PSUM"))

        # resident x slab: per partition 18 rows (16 + 2 halo) x 514 cols.
        # Split the load so the first group's rows land early (faster ramp).
        x_sb = x_pool.tile([NP, RB + 2, WP], FP32)
        nc.scalar.dma_start(out=x_sb[:, 0 : G + 2, :], in_=x_d[:, :, 0 : G + 2, :])
        nc.scalar.dma_start(
            out=x_sb[:, G + 2 : RB + 2, :], in_=x_d[:, :, G + 2 : RB + 2, :]
        )

        for grp in range(NGRP):
            R = grp * G
            out_sb = out_pool.tile([NP, G, W], FP32, tag="outsb")
            wts = []
            for half in range(2):
                w0 = half * HW
                # Stage this half-panel's 6-row x window into PSUM via the
                # (otherwise idle) ScalarE: the DVE multiplies then read x
                # through DVE's private PSUM port and weights through its
                # dedicated SBUF port, so the shared SBUF port pair stays
                # free and GpSimd's adds run truly in parallel.
                x_ps = xps_pool.tile([NP, G + 2, HWP], FP32, tag="xps")
                nc.scalar.copy(out=x_ps[:], in_=x_sb[:, R : R + G + 2, w0 : w0 + HWP])
                prods = []
                for t in range(KK):
                    i, j = t // K, t % K
                    if half == 0:
                        wt = w_pool.tile([NP, G, W], FP32, tag="wt")
                        nc.sync.dma_start(out=wt, in_=w_d[t, grp])
                        wts.append(wt)
                    wt = wts[t]
                    prod = prod_pool.tile([NP, G, HW], FP32, tag="prod")
                    nc.vector.tensor_tensor(
                        prod[:],
                        wt[:, :, w0 : w0 + HW],
                        x_ps[:, i : i + G, j : j + HW],
                        MULT,
                    )
                    prods.append(prod)

                # GpSimd folds taps 0..5 and the final combine (6 ops);
                # DVE folds taps 6..8 (2 ops).
                acc_p = acc_pool.tile([NP, G, HW], FP32, tag="accp")
                nc.gpsimd.tensor_tensor(acc_p[:], prods[0][:], prods[1][:], ADD)
                for t in range(2, 6):
                    nc.gpsimd.tensor_tensor(acc_p[:], acc_p[:], prods[t][:], ADD)
                acc_v = acc_pool.tile([NP, G, HW], FP32, tag="accv")
                nc.vector.tensor_tensor(acc_v[:], prods[6][:], prods[7][:], ADD)
                nc.vector.tensor_tensor(acc_v[:], acc_v[:], prods[8][:], ADD)
                nc.gpsimd.tensor_tensor(
                    out_sb[:, :, w0 : w0 + HW], acc_p[:], acc_v[:], ADD
                )
            nc.scalar.dma_start(out=o_d[grp], in_=out_sb[:])

    nc.compile()
    return nc


def _get_program() -> bacc.Bacc:
    global _PROGRAM
    if _PROGRAM is None:
        _PROGRAM = _build_program()
    return _PROGRAM


def _shard_inputs(input: np.ndarray, weight: np.ndarray) -> list[dict]:
    xp = np.pad(input, ((0, 0), (0, 0), (1, 1), (1, 1)))
    in_maps = []
    for k in range(N_CORES):
        b, hb = k // 4, k % 4
        h0 = hb * HL
        xs = xp[b, :, h0 : h0 + HL + 2, :]  # [C, 66, WP]
        # x: the HB overlapping 18-row windows -> [HB, C, 18, WP]
        x4 = np.stack(
            [xs[:, r0 : r0 + RB + 2, :] for r0 in range(0, HL, RB)]
        ).astype(np.float32)
        # weights: [C*KK, HL, W] -> [tap, grp, hb, c, r, w], contiguous per
        # (tap, grp) so each device DMA reads one linear 1 MiB block
        ws = (
            weight[b]
            .reshape(C, KK, H, W)[:, :, h0 : h0 + HL, :]
            .reshape(C, KK, HB, NGRP, G, W)
            .transpose(1, 3, 2, 0, 4, 5)
        )
        ws = np.ascontiguousarray(ws, dtype=np.float32)
        in_maps.append({"x": x4, "w": ws})
    return in_maps


def kernel(input: np.ndarray, weight: np.ndarray, _trace: bool = False):
    nc = _get_program()
    in_maps = _shard_inputs(np.asarray(input), np.asarray(weight))
    res = run_bass_kernel_spmd(
        nc, in_maps, core_ids=list(range(N_CORES)), trace=_trace
    )
    out = np.empty((B, C, H, W), dtype=np.float32)
    for k in range(N_CORES):
        b, hb = k // 4, k % 4
        # device out [grp, hb, c, r, w] -> [c, hb*16 + grp*4 + r, w]
        o = (
            res.results[k]["o"]
            .reshape(NGRP, HB, C, G, W)
            .transpose(2, 1, 0, 3, 4)
            .reshape(C, HL, W)
        )
        out[b, :, hb * HL : (hb + 1) * HL, :] = o
    if _trace:
        return out, res
    return out


# revision 15
# speedup vs baseline: 1.0569x; 1.0569x over previous
"""Per-pixel depthwise 3x3 conv (Conv2dLocal) on 8 Trainium2 NeuronCores.

out[b,c,h,w] = sum_{i,j in 3x3} x[b,c,h+i-1,w+j-1] * weight[b, c*9+3i+j, h, w]

Sharding: 8 cores = 2 batches x 4 H-slabs of 64 rows (data/spatial parallel).
The host pads the input spatially (1-px halo on H and W) and hands every core
an overlapping x slab, so the device program is identical and branch-free on
all cores (pure SPMD, no collectives).

Per-core layout: partition p = hb*32 + c (hb: 16-row block 0..3, c: channel);
free dim = (row, w), so all nine 3x3 tap shifts are free-dim offsets into a
single resident x slab [128, 18, 514].

DMA: the host pre-permutes the weight slab to [tap, grp, hb, c, r, w] so each
(tap, row-group) tile is one contiguous 1 MiB DRAM block — scattered-source
DMAs stripe over only 4 of the 16 SDMA engines (~110 GB/s), contiguous ones
use all 16 (~340 GB/s). Output uses a device-friendly contiguous layout too,
unscrambled on the host. Weights stream on the sync HWDGE ring; x and outputs
ride the scalar ring so they never queue behind the weight stream.

Compute: 9 DVE multiplies per group (fp32 tensor_tensor, 1x mode — uses only
DVE's dedicated SBUF port pair, so GpSimd runs truly in parallel) and the 8-way
tap accumulation split DVE/GpSimd. No PE/PSUM: fp32 matmul streams at 1/4 rate,
so identity-matmul accumulation is slower than DVE adds.
"""

import sys

if "/opt/trn_rl_repo" not in sys.path:
    sys.path.insert(0, "/opt/trn_rl_repo")

from contextlib import ExitStack

import numpy as np

import concourse.mybir as mybir
import concourse.tile as tile
from concourse import bacc
from concourse.bass_utils import run_bass_kernel_spmd
from concourse.masks import make_identity

# Problem shape (hardcoded per harness contract)
B, C, H, W = 2, 32, 256, 512
K = 3
KK = K * K
N_CORES = 8

# Per-core decomposition
HL = H // 4          # 64 local rows per core
HB = 4               # row-blocks per core (partition groups)
RB = HL // HB        # 16 rows per partition
G = 4                # rows processed per group
NGRP = RB // G       # 4 groups
WP = W + 2           # width incl. halo
NP = 128             # partitions

FP32 = mybir.dt.float32
ADD = mybir.AluOpType.add
MULT = mybir.AluOpType.mult

_PROGRAM = None


def _build_program() -> bacc.Bacc:
    nc = bacc.Bacc(
        "TRN2", target_bir_lowering=False, debug=False, num_devices=N_CORES
    )
    x_d = nc.declare_dram_parameter("x", [HB, C, RB + 2, WP], FP32, isOutput=False)
    w_d = nc.declare_dram_parameter(
        "w", [KK, NGRP, HB, C, G, W], FP32, isOutput=False
    )
    o_d = nc.declare_dram_parameter("o", [NGRP, HB, C, G, W], FP32, isOutput=True)

    with tile.TileContext(nc) as tc, ExitStack() as ctx:
        x_pool = ctx.enter_context(tc.tile_pool(name="x", bufs=1))
        w_pool = ctx.enter_context(tc.tile_pool(name="wt", bufs=8))
        prod_pool = ctx.enter_context(tc.tile_pool(name="prod", bufs=6))
        ape_pool = ctx.enter_context(tc.tile_pool(name="accpe", bufs=2))
        out_pool = ctx.enter_context(tc.tile_pool(name="outsb", bufs=2))
        const_pool = ctx.enter_context(tc.tile_pool(name="const", bufs=1))
        # group x window [128, 6, 514] fp32 = 7 PSUM banks; the 8th bank is
        # the PE's accumulation scratch
        xps_pool = ctx.enter_context(tc.tile_pool(name="xps", bufs=1, space="PSUM"))
        pe_pool = ctx.enter_context(tc.tile_pool(name="pe", bufs=1, space="PSUM"))

        ident = const_pool.tile([NP, NP], FP32)
        make_identity(nc, ident)

        # resident x slab: per partition 18 rows (16 + 2 halo) x 514 cols.
        # Split the load so the first group's rows land early (faster ramp).
        x_sb = x_pool.tile([NP, RB + 2, WP], FP32)
        nc.scalar.dma_start(out=x_sb[:, 0 : G + 2, :], in_=x_d[:, :, 0 : G + 2, :])
        nc.scalar.dma_start(
            out=x_sb[:, G + 2 : RB + 2, :], in_=x_d[:, :, G + 2 : RB + 2, :]
        )

        for grp in range(NGRP):
            R = grp * G
            # Stage this group's 6-row x window into PSUM via the idle
            # ScalarE: DVE multiplies then read x through DVE's private PSUM
            # port and weights through its dedicated SBUF port, so the
            # shared SBUF port pair stays free for GpSimd's adds.
            x_ps = xps_pool.tile([NP, G + 2, WP], FP32, tag="xps")
            nc.scalar.copy(out=x_ps[:], in_=x_sb[:, R : R + G + 2, :])
            prods = []
            for t in range(KK):
                i, j = t // K, t % K
                wt = w_pool.tile([NP, G, W], FP32, tag="wt")
                nc.sync.dma_start(out=wt, in_=w_d[t, grp])
                prod = prod_pool.tile([NP, G, W], FP32, tag="prod")
                nc.vector.tensor_tensor(
                    prod[:],
                    wt[:],
                    x_ps[:, i : i + G, j : j + W],
                    MULT,
                )
                prods.append(prod)

            # Tap accumulation split three ways:
            #  - PE: taps 0..2 via exact identity-matmul into the spare PSUM
            #    bank, one 512-col row-chunk at a time; the idle ScalarE
            #    ferries each chunk to SBUF (GpSimd cannot read PSUM).
            #  - GpSimd: taps 3..6 chained in place, then folds in the DVE
            #    partial and the PE partial.
            #  - DVE: taps 7..8.
            acc_pe = ape_pool.tile([NP, G, W], FP32, tag="accpe")
            for c in range(G):
                pe_ps = pe_pool.tile([NP, W], FP32, tag="peps")
                for t in range(3):
                    nc.tensor.matmul(
                        pe_ps[:],
                        ident[:],
                        prods[t][:, c, :],
                        start=(t == 0),
                        stop=(t == 2),
                    )
                nc.scalar.copy(out=acc_pe[:, c, :], in_=pe_ps[:])

            nc.vector.tensor_tensor(prods[7][:], prods[7][:], prods[8][:], ADD)

            a = prods[3]
            for t in range(4, 7):
                nc.gpsimd.tensor_tensor(a[:], a[:], prods[t][:], ADD)
            nc.gpsimd.tensor_tensor(a[:], a[:], prods[7][:], ADD)
            out_sb = out_pool.tile([NP, G, W], FP32, tag="outsb")
            nc.gpsimd.tensor_tensor(out_sb[:], a[:], acc_pe[:], ADD)
            nc.scalar.dma_start(out=o_d[grp], in_=out_sb[:])

    nc.compile()
    return nc


def _get_program() -> bacc.Bacc:
    global _PROGRAM
    if _PROGRAM is None:
        _PROGRAM = _build_program()
    return _PROGRAM


def _shard_inputs(input: np.ndarray, weight: np.ndarray) -> list[dict]:
    xp = np.pad(input, ((0, 0), (0, 0), (1, 1), (1, 1)))
    in_maps = []
    for k in range(N_CORES):
        b, hb = k // 4, k % 4
        h0 = hb * HL
        xs = xp[b, :, h0 : h0 + HL + 2, :]  # [C, 66, WP]
        # x: the HB overlapping 18-row windows -> [HB, C, 18, WP]
        x4 = np.stack(
            [xs[:, r0 : r0 + RB + 2, :] for r0 in range(0, HL, RB)]
        ).astype(np.float32)
        # weights: [C*KK, HL, W] -> [tap, grp, hb, c, r, w], contiguous per
        # (tap, grp) so each device DMA reads one linear 1 MiB block
        ws = (
            weight[b]
            .reshape(C, KK, H, W)[:, :, h0 : h0 + HL, :]
            .reshape(C, KK, HB, NGRP, G, W)
            .transpose(1, 3, 2, 0, 4, 5)
        )
        ws = np.ascontiguousarray(ws, dtype=np.float32)
        in_maps.append({"x": x4, "w": ws})
    return in_maps


def kernel(input: np.ndarray, weight: np.ndarray, _trace: bool = False):
    nc = _get_program()
    in_maps = _shard_inputs(np.asarray(input), np.asarray(weight))
    res = run_bass_kernel_spmd(
        nc, in_maps, core_ids=list(range(N_CORES)), trace=_trace
    )
    out = np.empty((B, C, H, W), dtype=np.float32)
    for k in range(N_CORES):
        b, hb = k // 4, k % 4
        # device out [grp, hb, c, r, w] -> [c, hb*16 + grp*4 + r, w]
        o = (
            res.results[k]["o"]
            .reshape(NGRP, HB, C, G, W)
            .transpose(2, 1, 0, 3, 4)
            .reshape(C, HL, W)
        )
        out[b, :, hb * HL : (hb + 1) * HL, :] = o
    if _trace:
        return out, res
    return out


# revision 16
# speedup vs baseline: 1.0726x; 1.0149x over previous
"""Per-pixel depthwise 3x3 conv (Conv2dLocal) on 8 Trainium2 NeuronCores.

out[b,c,h,w] = sum_{i,j in 3x3} x[b,c,h+i-1,w+j-1] * weight[b, c*9+3i+j, h, w]

Sharding: 8 cores = 2 batches x 4 H-slabs of 64 rows (data/spatial parallel).
The host pads the input spatially (1-px halo on H and W) and hands every core
an overlapping x slab, so the device program is identical and branch-free on
all cores (pure SPMD, no collectives).

Per-core layout: partition p = hb*32 + c (hb: 16-row block 0..3, c: channel);
free dim = (row, w), so all nine 3x3 tap shifts are free-dim offsets into a
single resident x slab [128, 18, 514].

DMA: the host pre-permutes the weight slab to [tap, grp, hb, c, r, w] so each
(tap, row-group) tile is one contiguous 1 MiB DRAM block — scattered-source
DMAs stripe over only 4 of the 16 SDMA engines (~110 GB/s), contiguous ones
use all 16 (~340 GB/s). Output uses a device-friendly contiguous layout too,
unscrambled on the host. Weights stream on the sync HWDGE ring; x and outputs
ride the scalar ring so they never queue behind the weight stream.

Compute: 9 DVE multiplies per group (fp32 tensor_tensor, 1x mode — uses only
DVE's dedicated SBUF port pair, so GpSimd runs truly in parallel) and the 8-way
tap accumulation split DVE/GpSimd. No PE/PSUM: fp32 matmul streams at 1/4 rate,
so identity-matmul accumulation is slower than DVE adds.
"""

import sys

if "/opt/trn_rl_repo" not in sys.path:
    sys.path.insert(0, "/opt/trn_rl_repo")

from contextlib import ExitStack

import numpy as np

import concourse.mybir as mybir
import concourse.tile as tile
from concourse import bacc
from concourse.bass_utils import run_bass_kernel_spmd
from concourse.masks import make_identity

# Problem shape (hardcoded per harness contract)
B, C, H, W = 2, 32, 256, 512
K = 3
KK = K * K
N_CORES = 8

# Per-core decomposition
HL = H // 4          # 64 local rows per core
HB = 4               # row-blocks per core (partition groups)
RB = HL // HB        # 16 rows per partition
G = 4                # rows processed per group
NGRP = RB // G       # 4 groups
WP = W + 2           # width incl. halo
NP = 128             # partitions

FP32 = mybir.dt.float32
ADD = mybir.AluOpType.add
MULT = mybir.AluOpType.mult

_PROGRAM = None


def _build_program() -> bacc.Bacc:
    nc = bacc.Bacc(
        "TRN2", target_bir_lowering=False, debug=False, num_devices=N_CORES
    )
    x_d = nc.declare_dram_parameter("x", [HB, C, RB + 2, WP], FP32, isOutput=False)
    w_d = nc.declare_dram_parameter(
        "w", [KK, NGRP, HB, C, G, W], FP32, isOutput=False
    )
    o_d = nc.declare_dram_parameter("o", [NGRP, HB, C, G, W], FP32, isOutput=True)

    with tile.TileContext(nc) as tc, ExitStack() as ctx:
        x_pool = ctx.enter_context(tc.tile_pool(name="x", bufs=1))
        w_pool = ctx.enter_context(tc.tile_pool(name="wt", bufs=6))
        prod_pool = ctx.enter_context(tc.tile_pool(name="prod", bufs=6))
        acc_pool = ctx.enter_context(tc.tile_pool(name="acc", bufs=2))
        ape_pool = ctx.enter_context(tc.tile_pool(name="accpe", bufs=2))
        out_pool = ctx.enter_context(tc.tile_pool(name="outsb", bufs=2))
        const_pool = ctx.enter_context(tc.tile_pool(name="const", bufs=1))
        # group x window [128, 6, 512] fp32 = exactly 6 PSUM banks (center
        # cols 1..512 of the 514-wide halo window); the remaining 2 banks
        # ping-pong the PE's accumulation scratch
        xps_pool = ctx.enter_context(tc.tile_pool(name="xps", bufs=1, space="PSUM"))
        pe_pool = ctx.enter_context(tc.tile_pool(name="pe", bufs=2, space="PSUM"))

        ident = const_pool.tile([NP, NP], FP32)
        make_identity(nc, ident)

        # resident x slab: per partition 18 rows (16 + 2 halo) x 514 cols.
        # Split the load so the first group's rows land early (faster ramp).
        x_sb = x_pool.tile([NP, RB + 2, WP], FP32)
        nc.scalar.dma_start(out=x_sb[:, 0 : G + 2, :], in_=x_d[:, :, 0 : G + 2, :])
        nc.scalar.dma_start(
            out=x_sb[:, G + 2 : RB + 2, :], in_=x_d[:, :, G + 2 : RB + 2, :]
        )

        for grp in range(NGRP):
            R = grp * G
            # Stage this group's 6-row x window (center 512 cols) into PSUM
            # via the idle ScalarE: DVE multiplies then read x through DVE's
            # private PSUM port and weights through its dedicated SBUF port,
            # so the shared SBUF port pair stays free for GpSimd's adds.
            # The 2 halo columns stay in SBUF and get tiny fix-up multiplies.
            x_ps = xps_pool.tile([NP, G + 2, W], FP32, tag="xps")
            nc.scalar.copy(out=x_ps[:], in_=x_sb[:, R : R + G + 2, 1 : 1 + W])
            prods = []
            for t in range(KK):
                i, j = t // K, t % K
                wt = w_pool.tile([NP, G, W], FP32, tag="wt")
                nc.sync.dma_start(out=wt, in_=w_d[t, grp])
                prod = prod_pool.tile([NP, G, W], FP32, tag="prod")
                if j == 1:
                    nc.vector.tensor_tensor(
                        prod[:], wt[:], x_ps[:, i : i + G, :], MULT
                    )
                elif j == 0:
                    # x window cols 0..511 = SBUF col 0 | PSUM cols 0..510
                    nc.vector.tensor_tensor(
                        prod[:, :, 1:W],
                        wt[:, :, 1:W],
                        x_ps[:, i : i + G, 0 : W - 1],
                        MULT,
                    )
                    nc.vector.tensor_tensor(
                        prod[:, :, 0:1],
                        wt[:, :, 0:1],
                        x_sb[:, R + i : R + i + G, 0:1],
                        MULT,
                    )
                else:
                    # x window cols 2..513 = PSUM cols 1..511 | SBUF col 513
                    nc.vector.tensor_tensor(
                        prod[:, :, 0 : W - 1],
                        wt[:, :, 0 : W - 1],
                        x_ps[:, i : i + G, 1:W],
                        MULT,
                    )
                    nc.vector.tensor_tensor(
                        prod[:, :, W - 1 : W],
                        wt[:, :, W - 1 : W],
                        x_sb[:, R + i : R + i + G, WP - 1 : WP],
                        MULT,
                    )
                prods.append(prod)

            # Tap accumulation split three ways:
            #  - GpSimd: early taps 0..3 chained into a dedicated acc tile
            #    (frees product slots early), then folds the DVE and PE
            #    partials.
            #  - PE: taps 4..6 via exact identity-matmul, one 512-col
            #    row-chunk at a time through the 2 ping-pong PSUM banks; the
            #    idle ScalarE ferries chunks to SBUF (GpSimd can't read PSUM).
            #  - DVE: taps 7..8.
            acc = acc_pool.tile([NP, G, W], FP32, tag="acc")
            nc.gpsimd.tensor_tensor(acc[:], prods[0][:], prods[1][:], ADD)
            nc.gpsimd.tensor_tensor(acc[:], acc[:], prods[2][:], ADD)
            nc.gpsimd.tensor_tensor(acc[:], acc[:], prods[3][:], ADD)

            acc_pe = ape_pool.tile([NP, G, W], FP32, tag="accpe")
            for c in range(G):
                pe_ps = pe_pool.tile([NP, W], FP32, tag="peps")
                for t in range(4, 7):
                    nc.tensor.matmul(
                        pe_ps[:],
                        ident[:],
                        prods[t][:, c, :],
                        start=(t == 4),
                        stop=(t == 6),
                    )
                nc.scalar.copy(out=acc_pe[:, c, :], in_=pe_ps[:])

            nc.vector.tensor_tensor(prods[7][:], prods[7][:], prods[8][:], ADD)

            nc.gpsimd.tensor_tensor(acc[:], acc[:], prods[7][:], ADD)
            out_sb = out_pool.tile([NP, G, W], FP32, tag="outsb")
            nc.gpsimd.tensor_tensor(out_sb[:], acc[:], acc_pe[:], ADD)
            nc.scalar.dma_start(out=o_d[grp], in_=out_sb[:])

    nc.compile()
    return nc


def _get_program() -> bacc.Bacc:
    global _PROGRAM
    if _PROGRAM is None:
        _PROGRAM = _build_program()
    return _PROGRAM


def _shard_inputs(input: np.ndarray, weight: np.ndarray) -> list[dict]:
    xp = np.pad(input, ((0, 0), (0, 0), (1, 1), (1, 1)))
    in_maps = []
    for k in range(N_CORES):
        b, hb = k // 4, k % 4
        h0 = hb * HL
        xs = xp[b, :, h0 : h0 + HL + 2, :]  # [C, 66, WP]
        # x: the HB overlapping 18-row windows -> [HB, C, 18, WP]
        x4 = np.stack(
            [xs[:, r0 : r0 + RB + 2, :] for r0 in range(0, HL, RB)]
        ).astype(np.float32)
        # weights: [C*KK, HL, W] -> [tap, grp, hb, c, r, w], contiguous per
        # (tap, grp) so each device DMA reads one linear 1 MiB block
        ws = (
            weight[b]
            .reshape(C, KK, H, W)[:, :, h0 : h0 + HL, :]
            .reshape(C, KK, HB, NGRP, G, W)
            .transpose(1, 3, 2, 0, 4, 5)
        )
        ws = np.ascontiguousarray(ws, dtype=np.float32)
        in_maps.append({"x": x4, "w": ws})
    return in_maps


def kernel(input: np.ndarray, weight: np.ndarray, _trace: bool = False):
    nc = _get_program()
    in_maps = _shard_inputs(np.asarray(input), np.asarray(weight))
    res = run_bass_kernel_spmd(
        nc, in_maps, core_ids=list(range(N_CORES)), trace=_trace
    )
    out = np.empty((B, C, H, W), dtype=np.float32)
    for k in range(N_CORES):
        b, hb = k // 4, k % 4
        # device out [grp, hb, c, r, w] -> [c, hb*16 + grp*4 + r, w]
        o = (
            res.results[k]["o"]
            .reshape(NGRP, HB, C, G, W)
            .transpose(2, 1, 0, 3, 4)
            .reshape(C, HL, W)
        )
        out[b, :, hb * HL : (hb + 1) * HL, :] = o
    if _trace:
        return out, res
    return out
